# revision 11
# baseline (speedup 1.0000x reference)
"""Masked dot-product attention on 8 Trainium2 NeuronCores.

Problem: q,k,v [64, 1024, 64] f32, valid_lens [64] int32.
  scores = q @ k^T / 8, mask keys >= valid_len to -1e6, softmax, @ v.

Per core: 8 batches, pure data parallelism, no collectives.  Host prep:
q,k pre-transposed to [D, S] (q also scaled by log2e/8 so device exps
are base-2), v rows past valid_len pre-zeroed with the 0/1 mask as a
65th column -- the masked softmax denominator falls out of the same
matmul as attn @ v.  Per-batch key tiles truncated to ceil(valid/128);
batches rank-sorted into 8 slots (one per core per slot, same baked
schedule everywhere).

The kernel is paced by exp throughput on TWO engines (ScalarE
1.11us/tile exact Exp; DVE ~1.8us/tile Schraudolph), everything else
is arranged to stay off their critical path:

  - DVE tiles: PE-assisted avg-2 Schraudolph, only TWO DVE ops/tile:
      ua = rint(x*1024 + C)      (PSUM f32 -> SBUF u16, full rate)
      ub = ua + 512              (int add, 2x mode, deferred)
    The u16 bit pattern read as f16 is 2^x with the mantissa linearly
    interpolating 2^frac; averaging two half-period phases cuts the
    sawtooth to ~+-0.85% rel.  Instead of averaging on the DVE, BOTH
    phases go into the attn@v accumulation as separate f16 weights: vm
    carries a second 65-column copy pre-scaled by sqrt(1/2), and the
    PE's PSUM f32 accumulation sums the phases for free (numerator and
    denominator alike -- the mask column is scaled too).  C folds the
    f16 exponent bias, the avg2 pre-halving, the sawtooth centering and
    the e^-3 exp bias, so ScalarE and DVE tiles mix freely within a
    batch.
  - NO on-device normalization: the [128, 4x65] PSUM accumulator banks
    (64 unnormalized output cols + denominator col per chunk) are
    DMA'd straight to DRAM as each bank closes, and the single
    divide-by-denominator runs on the host after gather.  This frees
    the DVE (and the tail) of the reciprocal+scale epilogue entirely.
  - PE p-state warm-up: the Tensor engine starts at ~1.2GHz and reaches
    2.4GHz only after ~3us of continuous work, and any idle resets it.
    A short dummy-matmul burst during the input-DMA dead window plus
    small top-of-slot bursts in the first batches ramp it early.
  - slot order [S3,S4,S5,S7,S6,S2,S1,S0] (ascending rank groups).
  - leftover attn@v of a batch is drained AFTER the next batch's score
    emission, never before it.
  - q/k/vm input DMAs: one replicated 3D-AP transfer each (q,k land in
    both partition halves for the row-group score pairs), halving the
    trigger count on the two DMA queues; inputs prefetched one slot
    ahead, kT dispatched before qT and qT column-split on slot 0;
    memsets run off the DMA-trigger queues so the first loads issue
    immediately.  The last slot runs its chunk groups bank-sequential
    and quarter-stores each bank as it closes for a short tail.
"""

import numpy as np

import concourse.bass as bass
import concourse.bacc as bacc
import concourse.tile as tile
from concourse import mybir
from concourse import bass_utils

B, S, D = 64, 1024, 64
NCORES = 8
NB = B // NCORES  # batch slots per core
P = 128
NJT = S // P  # max key tiles per batch
W = D + 1  # v columns + mask column
W2 = 2 * W  # vm carries both avg-2 phases side by side
F32 = mybir.dt.float32
F16 = mybir.dt.float16
U16 = mybir.dt.uint16

LN2 = float(np.log(2.0))
LOG2E = float(np.log2(np.e))
SCH_CORR = 55.0
# u16 = x*1024 + C; C folds the f16 bias (15*1024), the avg2 pre-halving
# (-1024), the sawtooth centering, and the exp bias e^-3 (ScalarE tiles
# use bias=-3, so DVE tiles fold -3*log2e into the exponent domain).
SCH_C = float(15 * 1024 - 1024 - SCH_CORR - 3.0 * LOG2E * 1024)
SQRT_HALF = float(2.0 ** -0.5)

# fraction of each batch's tiles routed to the DVE Schraudolph path
DVE_FRAC = 0.30

TRACE = False  # set by test harness to capture an NTFF profile
LAST_RESULTS = None  # BassKernelResults stash for the harness

_program_cache = {}


def _av_steps(nc, po_pool, ocp_pool, out, s, jt, exs, vm_t3, last=False):
    """Yield one emission step at a time: 8 attn@v chunk-groups
    accumulating into one [128, 2x512] PSUM tile (two 2KB banks; group h
    occupies cols [h*512, h*512+260): 4 chunks of 65 = 64 unnormalized
    output cols + denominator col, bank-aligned so no chunk crosses a
    bank).  The caller interleaves these steps between the NEXT batch's
    score/exp pairs so the PE queue alternates between feeding the exp
    engines (scores) and draining them (attn@v).

    exs[j] is a tuple of (weight, phase) pairs: one (ex, 0) for exact-exp
    tiles, or ((ua, 0), (ub, 1)) for Schraudolph tiles whose two phases
    accumulate against vm columns [0:65] and [65:130] (the latter
    pre-scaled by sqrt(1/2) on the host).

    When the slot's 8 groups close, ONE DVE copy moves both 260-col
    groups PSUM->SBUF and one DMA stores the raw numerator+denominator
    rows; the divide is on the host.  The last slot runs bank-sequential
    with a copy + two quarter-stores per bank as it closes, for a short
    tail.
    """
    po = po_pool.tile([P, 2 * 512], F32, tag="po", name="po")
    po_r = po.rearrange("p (h x) -> p h x", x=512)
    order = [0, 1, 2, 3, 4, 5, 6, 7] if last else [0, 4, 1, 5, 2, 6, 3, 7]
    ocp = ocp_pool.tile([P, 8 * W], F32, tag="ocp", name="ocp")
    ocp_r = ocp.rearrange("p (h c w) -> p h (c w)", h=2, w=W)
    nmm = sum(len(e) for e in exs)  # matmuls per chunk group
    for qc in order:
        h = qc // 4
        col = h * 512 + (qc % 4) * W
        mi = 0
        for j in range(jt):
            for wgt, ph in exs[j]:
                nc.tensor.matmul(
                    po[:, col:col + W],
                    lhsT=wgt[:, qc * P:(qc + 1) * P],
                    rhs=vm_t3[:, j, ph * W:(ph + 1) * W],
                    start=(mi == 0), stop=(mi == nmm - 1),
                )
                mi += 1
                # fine-grained steps: never queue more than ~4 attn@v
                # matmuls ahead of the next batch's scores, or the exp
                # engines starve
                if mi % 4 == 0:
                    yield
        if last and qc in (3, 7):
            # tail: copy + two ~66KB quarter stores per bank as it closes
            nc.vector.tensor_scalar(
                out=ocp_r[:, h], in0=po_r[:, h, 0:4 * W], scalar1=0.0,
                scalar2=None, op0=mybir.AluOpType.add)
            for qq in range(2):
                c0 = h * 4 + qq * 2
                eng = nc.gpsimd if qq == 0 else nc.sync
                eng.dma_start(
                    out=out[s, c0 * P:(c0 + 2) * P].rearrange(
                        "(c p) w -> p c w", p=P),
                    in_=ocp.rearrange("p (c w) -> p c w", w=W)[:, c0:c0 + 2],
                )
        elif not last and qc == order[-1]:
            # slot complete: one PSUM->SBUF copy of both groups, one store
            nc.vector.tensor_scalar(
                out=ocp_r, in0=po_r[:, :, 0:4 * W], scalar1=0.0,
                scalar2=None, op0=mybir.AluOpType.add)
            eng = nc.gpsimd if s % 2 == 0 else nc.sync
            eng.dma_start(
                out=out[s].rearrange("(c p) w -> p c w", p=P),
                in_=ocp.rearrange("p (c w) -> p c w", w=W),
            )
        yield


def _build_program(slots):
    """slots: tuple of (jt, ndve) per batch slot."""
    nc = bacc.Bacc("TRN2", target_bir_lowering=False, debug=False,
                   num_devices=NCORES)
    qT = nc.dram_tensor("qT", [NB, D, S], F16, kind="ExternalInput").ap()
    kT = nc.dram_tensor("kT", [NB, D, S], F16, kind="ExternalInput").ap()
    vm = nc.dram_tensor("vm", [NB, S, W2], F16, kind="ExternalInput").ap()
    out = nc.dram_tensor("out", [NB, S, W], F32, kind="ExternalOutput").ap()

    sum_jt = sum(jt for jt, _ in slots)
    sum_nd = sum(nd for _, nd in slots)

    with tile.TileContext(nc) as tc:
        with (
            tc.tile_pool(name="singles", bufs=1) as singles,
            tc.tile_pool(name="qk", bufs=3) as qk_pool,
            tc.tile_pool(name="vmp", bufs=4) as vm_pool,
            tc.tile_pool(name="ex", bufs=max(sum_jt - sum_nd, 1)) as ex_pool,
            tc.tile_pool(name="ua", bufs=max(sum_nd, 1)) as ua_pool,
            tc.tile_pool(name="ub", bufs=max(sum_nd, 1)) as ub_pool,
            tc.tile_pool(name="ocp", bufs=3) as ocp_pool,
            tc.tile_pool(name="ps_s", bufs=2, space="PSUM") as ps_pool,
            tc.tile_pool(name="ps_o", bufs=2, space="PSUM") as po_pool,
        ):
            # ScalarE tiles: exp(x*ln2 - 3) on x = qk*log2e/8 (qT
            # pre-scaled); -3 bounds the fp16 exp range and cancels
            # between numerator and denominator.  Memsets run on engines
            # OFF the DMA-trigger queues so the first input DMAs issue
            # immediately.
            bias_t = singles.tile([P, 1], F32)
            nc.vector.memset(bias_t, -3.0)

            # PE warm-up: ramp the p-state during the input-DMA dead
            # window; idle resets it, so early slots add small bursts.
            warm = singles.tile([P, 512], F16)
            nc.vector.memset(warm, 1.0)

            def warmup(n):
                wps = ps_pool.tile([P, S], F32, tag="ps", name="wps")
                for _ in range(n):
                    nc.tensor.matmul(wps[:, 0:512], lhsT=warm[:, 0:P],
                                     rhs=warm, start=True, stop=True)

            def emit_input_dmas(s, jt, first=False):
                # q/k replicated into both partition halves with a single
                # 3D-AP DMA each (0-stride outer source dim) so score
                # matmuls for two key-tiles can run concurrently on PE
                # row-groups (0..63) and (64..127).  kT before qT
                # (LDWEIGHTS gates first); on the first slot qT is
                # column-split so the first score pair starts early.
                qT_t = qk_pool.tile([2 * D, S], F16, tag="qT", name="qT_t")
                kT_t = qk_pool.tile([2 * D, S], F16, tag="kT", name="kT_t")
                nc.sync.dma_start(out=kT_t[0:D, 0:jt * P],
                                  in_=kT[s, :, 0:jt * P])
                nc.gpsimd.dma_start(out=kT_t[D:2 * D, 0:jt * P],
                                    in_=kT[s, :, 0:jt * P])
                if first:
                    nc.sync.dma_start(out=qT_t[0:D, 0:512],
                                      in_=qT[s, :, 0:512])
                    nc.gpsimd.dma_start(out=qT_t[D:2 * D, 0:512],
                                        in_=qT[s, :, 0:512])
                    nc.sync.dma_start(out=qT_t[0:D, 512:S],
                                      in_=qT[s, :, 512:S])
                    nc.gpsimd.dma_start(out=qT_t[D:2 * D, 512:S],
                                        in_=qT[s, :, 512:S])
                else:
                    nc.sync.dma_start(out=qT_t[0:D, :], in_=qT[s])
                    nc.gpsimd.dma_start(out=qT_t[D:2 * D, :], in_=qT[s])
                # All key tiles of vm in one DMA: [128, jt*130], tile j at
                # columns [j*130, (j+1)*130).
                vm_t = vm_pool.tile([P, NJT * W2], F16, tag="vm", name="vm_t")
                nc.sync.dma_start(
                    out=vm_t.rearrange("p (j w) -> p j w", w=W2)[:, 0:jt, :],
                    in_=vm[s, 0:jt * P, :].rearrange("(j p) w -> p j w", p=P),
                )
                return qT_t, kT_t, vm_t

            from collections import deque
            pending = deque()  # unfinished attn@v/store generators
            drip = 1
            ub_backlog = []
            staged = emit_input_dmas(0, slots[0][0], first=True)
            warmup(3)
            for s, (jt, ndve) in enumerate(slots):
                qT_t, kT_t, vm_t = staged
                vm_t3 = vm_t.rearrange("p (j w) -> p j w", w=W2)
                if 1 <= s <= 3:
                    warmup(2)
                if s + 1 < NB:
                    # prefetch the next slot's inputs one slot ahead so
                    # its first score pair never waits on the DMA queue
                    staged = emit_input_dmas(s + 1, slots[s + 1][0])
                # Score matmuls go out in row-group-interleaved pairs --
                # adjacent PE-queue entries on disjoint row groups execute
                # concurrently, so a pair of key tiles costs one tile's time.
                exs = []
                for m in range(0, jt, 2):
                    js = list(range(m, min(m + 2, jt)))
                    pss = [ps_pool.tile([P, S], F32, tag="ps", name="ps")
                           for _ in js]
                    for half in range(2):
                        for r, j in enumerate(js):
                            nc.tensor.matmul(
                                pss[r][:, half * 512:(half + 1) * 512],
                                lhsT=kT_t[r * D:(r + 1) * D,
                                          j * P:(j + 1) * P],
                                rhs=qT_t[r * D:(r + 1) * D,
                                         half * 512:(half + 1) * 512],
                                start=True, stop=True,
                                tile_position=(r * D, 0),
                            )
                    for r, j in enumerate(js):
                        if j < ndve:
                            # emit only the PSUM-reading op now so the
                            # score buffer frees as fast as an ACTIVATE
                            # would; the SBUF-only +512 phase op is
                            # deferred to the end of the batch's scores.
                            ua = ua_pool.tile([P, S], U16, tag="ua",
                                              name="ua")
                            ub = ub_pool.tile([P, S], U16, tag="ub",
                                              name="ub")
                            nc.vector.tensor_scalar(
                                out=ua, in0=pss[r], scalar1=1024.0,
                                scalar2=SCH_C, op0=mybir.AluOpType.mult,
                                op1=mybir.AluOpType.add)
                            ub_backlog.append((ua, ub))
                            exs.append(((ua.bitcast(F16), 0),
                                        (ub.bitcast(F16), 1)))
                        else:
                            ex = ex_pool.tile([P, S], F16, tag="ex",
                                              name="ex")
                            nc.scalar.activation(
                                out=ex, in_=pss[r],
                                func=mybir.ActivationFunctionType.Exp,
                                scale=LN2, bias=bias_t)
                            exs.append(((ex, 0),))
                        # drain a sliver of the pending attn@v stream
                        # after each exp (keeps the exp engines and PE both
                        # fed), paced to finish just before this batch's
                        # own attn@v
                        for _ in range(drip):
                            if not pending:
                                break
                            if next(pending[0], "done") == "done":
                                pending.popleft()
                for ua, ub in ub_backlog:
                    nc.vector.tensor_scalar(
                        out=ub, in0=ua, scalar1=512, scalar2=None,
                        op0=mybir.AluOpType.add)
                ub_backlog = []
                # drain any leftover attn@v now -- after this batch's
                # scores, so it never blocks them on the PE
                while pending:
                    for _ in pending.popleft():
                        pass
                pending.append(
                    _av_steps(nc, po_pool, ocp_pool, out, s, jt, exs, vm_t3,
                              last=(s == NB - 1)))
                nsteps = 8 * ((jt + ndve) // 4 + 1)
                nxt = slots[s + 1][0] if s + 1 < NB else jt
                drip = max(1, -(-nsteps // max(2 * nxt, 1)))
            for gen in pending:
                for _ in gen:
                    pass
    nc.compile()
    return nc


def kernel(q, k, v, valid_lens):
    global LAST_RESULTS
    q = np.array(q, dtype=np.float32, copy=True)
    k = np.asarray(k, dtype=np.float32)
    v = np.asarray(v, dtype=np.float32)
    vl = np.asarray(valid_lens).astype(np.int64)

    # valid_len == 0: reference's softmax over an all-masked row is uniform.
    # Zeroed q gives scores == 0 -> exp == 1 over all (unmasked) keys: same.
    valid_eff = np.where(vl <= 0, S, np.minimum(vl, S))
    q[vl <= 0] = 0.0

    mask = (np.arange(S)[None, :] < valid_eff[:, None]).astype(np.float32)
    # qT carries the 1/8 score scale and log2e: scores become x = s*log2e/8,
    # so exp(s/8) = 2^x for both exp engines.
    qT = np.ascontiguousarray(q.transpose(0, 2, 1) * np.float32(LOG2E / 8))
    qT = qT.astype(np.float16)
    kT = np.ascontiguousarray(k.transpose(0, 2, 1)).astype(np.float16)
    vm1 = np.concatenate([v * mask[:, :, None], mask[:, :, None]], axis=2)
    # second copy pre-scaled by sqrt(1/2): the avg-2 phase-b weight
    # accumulates against it, so the PE sums the two Schraudolph phases.
    vmh = np.concatenate([vm1, vm1 * np.float32(SQRT_HALF)], axis=2)
    vmh = np.ascontiguousarray(vmh).astype(np.float16)

    # Rank-sort batches; slot s takes one batch of rank group [8s, 8s+8)
    # per core, so the baked per-slot tile count wastes little work.
    order = np.argsort(-valid_eff, kind="stable")
    groups = order.reshape(NB, NCORES)[::-1]  # ascending tile counts
    # [S3, S4, S5, S7, S6, S2, S1, S0]: medium slot first (enough exp
    # work to cover the next slot's DMA), the largest batches mid-kernel
    # so the PE gets long continuous stretches (p-state ramp), the two
    # smallest batches last for a short tail.
    perm = [3, 4, 5, NB - 1, NB - 2, 2, 1, 0]
    groups = groups[perm]
    jt_counts = [int(np.ceil(valid_eff[groups[s]].max() / P))
                 for s in range(NB)]
    slots = tuple((jt, int(jt * DVE_FRAC + 0.5))
                  for jt in jt_counts)

    nc = _program_cache.get(slots)
    if nc is None:
        nc = _build_program(slots)
        _program_cache[slots] = nc

    in_maps = []
    for c in range(NCORES):
        bs = groups[:, c]
        in_maps.append({
            "qT": np.ascontiguousarray(qT[bs]),
            "kT": np.ascontiguousarray(kT[bs]),
            "vm": np.ascontiguousarray(vmh[bs]),
        })
    res = bass_utils.run_bass_kernel_spmd(
        nc, in_maps, core_ids=list(range(NCORES)), trace=TRACE,
    )
    LAST_RESULTS = res

    out = np.empty((B, S, D), dtype=np.float32)
    for c in range(NCORES):
        o = res.results[c]["out"]  # [NB, S, W]: numerator + denominator
        on = o[:, :, 0:D] / o[:, :, D:D + 1]
        for s in range(NB):
            out[groups[s, c]] = on[s]
    return out


# revision 13
# speedup vs baseline: 1.1672x; 1.1672x over previous
"""Masked dot-product attention on 8 Trainium2 NeuronCores.

Problem: q,k,v [64, 1024, 64] f32, valid_lens [64] int32.
  scores = q @ k^T / 8, mask keys >= valid_len to -1e6, softmax, @ v.

Per core: 8 batches, pure data parallelism, no collectives.  Host prep:
q,k pre-transposed to [D, S] (q also scaled by log2e/8 so device exps
are base-2), v rows past valid_len pre-zeroed with the 0/1 mask as a
65th column -- the masked softmax denominator falls out of the same
matmul as attn @ v.  Per-batch key tiles truncated to ceil(valid/128);
batches rank-sorted into 8 slots (one per core per slot, same baked
schedule everywhere).

The kernel is paced by exp throughput on TWO engines (ScalarE
1.11us/tile exact Exp; DVE ~1.8us/tile Schraudolph), everything else
is arranged to stay off their critical path:

  - DVE tiles: PE-assisted avg-2 Schraudolph, only TWO DVE ops/tile:
      ua = rint(x*1024 + C)      (PSUM f32 -> SBUF u16, full rate)
      ub = ua + 512              (int add, 2x mode, deferred)
    The u16 bit pattern read as f16 is 2^x with the mantissa linearly
    interpolating 2^frac; averaging two half-period phases cuts the
    sawtooth to ~+-0.85% rel.  Instead of averaging on the DVE, BOTH
    phases go into the attn@v accumulation as separate f16 weights: vm
    carries a second 65-column copy pre-scaled by sqrt(1/2), and the
    PE's PSUM f32 accumulation sums the phases for free (numerator and
    denominator alike -- the mask column is scaled too).  C folds the
    f16 exponent bias, the avg2 pre-halving, the sawtooth centering and
    the e^-3 exp bias, so ScalarE and DVE tiles mix freely within a
    batch.
  - NO on-device normalization: the [128, 4x65] PSUM accumulator banks
    (64 unnormalized output cols + denominator col per chunk) are
    DMA'd straight to DRAM as each bank closes, and the single
    divide-by-denominator runs on the host after gather.  This frees
    the DVE (and the tail) of the reciprocal+scale epilogue entirely.
  - PE p-state warm-up: the Tensor engine starts at ~1.2GHz and reaches
    2.4GHz only after ~3us of continuous work, and any idle resets it.
    A short dummy-matmul burst during the input-DMA dead window plus
    small top-of-slot bursts in the first batches ramp it early.
  - slot order [S3,S4,S5,S7,S6,S2,S1,S0] (ascending rank groups).
  - leftover attn@v of a batch is drained AFTER the next batch's score
    emission, never before it.
  - q/k/vm input DMAs: one replicated 3D-AP transfer each (q,k land in
    both partition halves for the row-group score pairs), halving the
    trigger count on the two DMA queues; inputs prefetched one slot
    ahead, kT dispatched before qT and qT column-split on slot 0;
    memsets run off the DMA-trigger queues so the first loads issue
    immediately.  The last slot runs its chunk groups bank-sequential
    and quarter-stores each bank as it closes for a short tail.
"""

import numpy as np

import concourse.bass as bass
import concourse.bacc as bacc
import concourse.tile as tile
from concourse import mybir
from concourse import bass_utils

B, S, D = 64, 1024, 64
NCORES = 8
NB = B // NCORES  # batch slots per core
P = 128
NJT = S // P  # max key tiles per batch
W = D + 1  # v columns + mask column
W2 = 2 * W  # vm carries both avg-2 phases side by side
F32 = mybir.dt.float32
F16 = mybir.dt.float16
U16 = mybir.dt.uint16

LN2 = float(np.log(2.0))
LOG2E = float(np.log2(np.e))
SCH_CORR = 55.0
# u16 = x*1024 + C; C folds the f16 bias (15*1024), the avg2 pre-halving
# (-1024), the sawtooth centering, and the exp bias e^-3 (ScalarE tiles
# use bias=-3, so DVE tiles fold -3*log2e into the exponent domain).
SCH_C = float(15 * 1024 - 1024 - SCH_CORR - 3.0 * LOG2E * 1024)
SQRT_HALF = float(2.0 ** -0.5)

# fraction of each batch's tiles routed to the DVE Schraudolph path
DVE_FRAC = 0.30

TRACE = False  # set by test harness to capture an NTFF profile
LAST_RESULTS = None  # BassKernelResults stash for the harness

_program_cache = {}


def _av_steps(nc, po_pool, ocp_pool, out, s, jt, exs, vm_t3, last=False):
    """Yield one emission step at a time: 8 attn@v chunk-groups
    accumulating into one [128, 2x512] PSUM tile (two 2KB banks; group h
    occupies cols [h*512, h*512+260): 4 chunks of 65 = 64 unnormalized
    output cols + denominator col, bank-aligned so no chunk crosses a
    bank).  The caller interleaves these steps between the NEXT batch's
    score/exp pairs so the PE queue alternates between feeding the exp
    engines (scores) and draining them (attn@v).

    exs[j] is a tuple of (weight, phase) pairs: one (ex, 0) for exact-exp
    tiles, or ((ua, 0), (ub, 1)) for Schraudolph tiles whose two phases
    accumulate against vm columns [0:65] and [65:130] (the latter
    pre-scaled by sqrt(1/2) on the host).

    When the slot's 8 groups close, ONE DVE copy moves both 260-col
    groups PSUM->SBUF and one DMA stores the raw numerator+denominator
    rows; the divide is on the host.  The last slot runs bank-sequential
    with a copy + two quarter-stores per bank as it closes, for a short
    tail.
    """
    po = po_pool.tile([P, 2 * 512], F32, tag="po", name="po")
    po_r = po.rearrange("p (h x) -> p h x", x=512)
    ocp = ocp_pool.tile([P, 8 * W], F32, tag="ocp", name="ocp")
    ocp_r = ocp.rearrange("p (h c w) -> p h (c w)", h=2, w=W)
    nmm = sum(len(e) for e in exs)  # matmuls per chunk group
    for qc in range(8):
        h = qc // 4
        col = h * 512 + (qc % 4) * W
        mi = 0
        for j in range(jt):
            for wgt, ph in exs[j]:
                nc.tensor.matmul(
                    po[:, col:col + W],
                    lhsT=wgt[:, qc * P:(qc + 1) * P],
                    rhs=vm_t3[:, j, ph * W:(ph + 1) * W],
                    start=(mi == 0), stop=(mi == nmm - 1),
                )
                mi += 1
                # fine-grained steps: never queue more than ~4 attn@v
                # matmuls ahead of the next batch's scores, or the exp
                # engines starve
                if mi % 4 == 0:
                    yield
        if qc in (3, 7):
            # bank h's 4 chunk groups all closed: copy it PSUM->SBUF and
            # store its half while the other bank is still accumulating
            # (this is also what lets the single po tile recycle without
            # stalling the next slot's attn@v).
            nc.vector.tensor_scalar(
                out=ocp_r[:, h], in0=po_r[:, h, 0:4 * W], scalar1=0.0,
                scalar2=None, op0=mybir.AluOpType.add)
            if last:
                # tail: two ~66KB quarter stores on both queues
                for qq in range(2):
                    c0 = h * 4 + qq * 2
                    eng = nc.gpsimd if qq == 0 else nc.sync
                    eng.dma_start(
                        out=out[s, c0 * P:(c0 + 2) * P].rearrange(
                            "(c p) w -> p c w", p=P),
                        in_=ocp.rearrange(
                            "p (c w) -> p c w", w=W)[:, c0:c0 + 2],
                    )
            else:
                eng = nc.gpsimd if h == 0 else nc.sync
                eng.dma_start(
                    out=out[s, h * 4 * P:(h + 1) * 4 * P].rearrange(
                        "(c p) w -> p c w", p=P),
                    in_=ocp.rearrange(
                        "p (c w) -> p c w", w=W)[:, h * 4:h * 4 + 4],
                )
        yield


def _build_program(slots):
    """slots: tuple of (jt, ndve) per batch slot."""
    nc = bacc.Bacc("TRN2", target_bir_lowering=False, debug=False,
                   num_devices=NCORES)
    qT = nc.dram_tensor("qT", [NB, D, S], F16, kind="ExternalInput").ap()
    kT = nc.dram_tensor("kT", [NB, D, S], F16, kind="ExternalInput").ap()
    vm = nc.dram_tensor("vm", [NB, S, W2], F16, kind="ExternalInput").ap()
    out = nc.dram_tensor("out", [NB, S, W], F32, kind="ExternalOutput").ap()

    sum_jt = sum(jt for jt, _ in slots)
    sum_nd = sum(nd for _, nd in slots)

    with tile.TileContext(nc) as tc:
        with (
            tc.tile_pool(name="singles", bufs=1) as singles,
            tc.tile_pool(name="qk", bufs=3) as qk_pool,
            tc.tile_pool(name="vmp", bufs=4) as vm_pool,
            tc.tile_pool(name="ex", bufs=max(sum_jt - sum_nd, 1)) as ex_pool,
            tc.tile_pool(name="ua", bufs=max(sum_nd, 1)) as ua_pool,
            tc.tile_pool(name="ub", bufs=max(sum_nd, 1)) as ub_pool,
            tc.tile_pool(name="ocp", bufs=3) as ocp_pool,
            tc.tile_pool(name="ps_s", bufs=3, space="PSUM") as ps_pool,
            tc.tile_pool(name="ps_o", bufs=1, space="PSUM") as po_pool,
        ):
            # ScalarE tiles: exp(x*ln2 - 3) on x = qk*log2e/8 (qT
            # pre-scaled); -3 bounds the fp16 exp range and cancels
            # between numerator and denominator.  Memsets run on engines
            # OFF the DMA-trigger queues so the first input DMAs issue
            # immediately.
            bias_t = singles.tile([P, 1], F32)
            nc.vector.memset(bias_t, -3.0)

            # PE warm-up: ramp the p-state during the input-DMA dead
            # window; idle resets it, so early slots add small bursts.
            warm = singles.tile([P, 512], F16)
            nc.vector.memset(warm, 1.0)

            def warmup(n):
                wps = ps_pool.tile([P, S], F32, tag="ps", name="wps")
                for _ in range(n):
                    nc.tensor.matmul(wps[:, 0:512], lhsT=warm[:, 0:P],
                                     rhs=warm, start=True, stop=True)

            def emit_input_dmas(s, jt, first=False):
                # q/k replicated into both partition halves with a single
                # 3D-AP DMA each (0-stride outer source dim) so score
                # matmuls for two key-tiles can run concurrently on PE
                # row-groups (0..63) and (64..127).  kT before qT
                # (LDWEIGHTS gates first); on the first slot qT is
                # column-split so the first score pair starts early.
                qT_t = qk_pool.tile([2 * D, S], F16, tag="qT", name="qT_t")
                kT_t = qk_pool.tile([2 * D, S], F16, tag="kT", name="kT_t")
                nc.sync.dma_start(out=kT_t[0:D, 0:jt * P],
                                  in_=kT[s, :, 0:jt * P])
                nc.gpsimd.dma_start(out=kT_t[D:2 * D, 0:jt * P],
                                    in_=kT[s, :, 0:jt * P])
                if first:
                    nc.sync.dma_start(out=qT_t[0:D, 0:512],
                                      in_=qT[s, :, 0:512])
                    nc.gpsimd.dma_start(out=qT_t[D:2 * D, 0:512],
                                        in_=qT[s, :, 0:512])
                    nc.sync.dma_start(out=qT_t[0:D, 512:S],
                                      in_=qT[s, :, 512:S])
                    nc.gpsimd.dma_start(out=qT_t[D:2 * D, 512:S],
                                        in_=qT[s, :, 512:S])
                else:
                    nc.sync.dma_start(out=qT_t[0:D, :], in_=qT[s])
                    nc.gpsimd.dma_start(out=qT_t[D:2 * D, :], in_=qT[s])
                # All key tiles of vm in one DMA: [128, jt*130], tile j at
                # columns [j*130, (j+1)*130).
                vm_t = vm_pool.tile([P, NJT * W2], F16, tag="vm", name="vm_t")
                nc.sync.dma_start(
                    out=vm_t.rearrange("p (j w) -> p j w", w=W2)[:, 0:jt, :],
                    in_=vm[s, 0:jt * P, :].rearrange("(j p) w -> p j w", p=P),
                )
                return qT_t, kT_t, vm_t

            from collections import deque
            pending = deque()  # unfinished attn@v/store generators
            drip = 1
            ub_backlog = []
            staged = emit_input_dmas(0, slots[0][0], first=True)
            warmup(3)
            for s, (jt, ndve) in enumerate(slots):
                qT_t, kT_t, vm_t = staged
                vm_t3 = vm_t.rearrange("p (j w) -> p j w", w=W2)
                if 1 <= s <= 3:
                    warmup(2)
                if s + 1 < NB:
                    # prefetch the next slot's inputs one slot ahead so
                    # its first score pair never waits on the DMA queue
                    staged = emit_input_dmas(s + 1, slots[s + 1][0])
                # Score matmuls go out in row-group-interleaved pairs --
                # adjacent PE-queue entries on disjoint row groups execute
                # concurrently, so a pair of key tiles costs one tile's time.
                exs = []
                for m in range(0, jt, 2):
                    js = list(range(m, min(m + 2, jt)))
                    pss = [ps_pool.tile([P, S], F32, tag="ps", name="ps")
                           for _ in js]
                    for half in range(2):
                        for r, j in enumerate(js):
                            nc.tensor.matmul(
                                pss[r][:, half * 512:(half + 1) * 512],
                                lhsT=kT_t[r * D:(r + 1) * D,
                                          j * P:(j + 1) * P],
                                rhs=qT_t[r * D:(r + 1) * D,
                                         half * 512:(half + 1) * 512],
                                start=True, stop=True,
                                tile_position=(r * D, 0),
                            )
                    for r, j in enumerate(js):
                        if j < ndve:
                            # emit only the PSUM-reading op now so the
                            # score buffer frees as fast as an ACTIVATE
                            # would; the SBUF-only +512 phase op is
                            # deferred to the end of the batch's scores.
                            ua = ua_pool.tile([P, S], U16, tag="ua",
                                              name="ua")
                            ub = ub_pool.tile([P, S], U16, tag="ub",
                                              name="ub")
                            nc.vector.tensor_scalar(
                                out=ua, in0=pss[r], scalar1=1024.0,
                                scalar2=SCH_C, op0=mybir.AluOpType.mult,
                                op1=mybir.AluOpType.add)
                            ub_backlog.append((ua, ub))
                            exs.append(((ua.bitcast(F16), 0),
                                        (ub.bitcast(F16), 1)))
                        else:
                            ex = ex_pool.tile([P, S], F16, tag="ex",
                                              name="ex")
                            nc.scalar.activation(
                                out=ex, in_=pss[r],
                                func=mybir.ActivationFunctionType.Exp,
                                scale=LN2, bias=bias_t)
                            exs.append(((ex, 0),))
                        # drain a sliver of the pending attn@v stream
                        # after each exp (keeps the exp engines and PE both
                        # fed), paced to finish just before this batch's
                        # own attn@v
                        for _ in range(drip):
                            if not pending:
                                break
                            if next(pending[0], "done") == "done":
                                pending.popleft()
                for ua, ub in ub_backlog:
                    nc.vector.tensor_scalar(
                        out=ub, in0=ua, scalar1=512, scalar2=None,
                        op0=mybir.AluOpType.add)
                ub_backlog = []
                # drain any leftover attn@v now -- after this batch's
                # scores, so it never blocks them on the PE
                while pending:
                    for _ in pending.popleft():
                        pass
                pending.append(
                    _av_steps(nc, po_pool, ocp_pool, out, s, jt, exs, vm_t3,
                              last=(s == NB - 1)))
                nsteps = 8 * ((jt + ndve) // 4 + 1)
                nxt = slots[s + 1][0] if s + 1 < NB else jt
                drip = max(1, -(-nsteps // max(2 * nxt, 1)))
            for gen in pending:
                for _ in gen:
                    pass
    nc.compile()
    return nc


def kernel(q, k, v, valid_lens):
    global LAST_RESULTS
    q = np.array(q, dtype=np.float32, copy=True)
    k = np.asarray(k, dtype=np.float32)
    v = np.asarray(v, dtype=np.float32)
    vl = np.asarray(valid_lens).astype(np.int64)

    # valid_len == 0: reference's softmax over an all-masked row is uniform.
    # Zeroed q gives scores == 0 -> exp == 1 over all (unmasked) keys: same.
    valid_eff = np.where(vl <= 0, S, np.minimum(vl, S))
    q[vl <= 0] = 0.0

    mask = (np.arange(S)[None, :] < valid_eff[:, None]).astype(np.float32)
    # qT carries the 1/8 score scale and log2e: scores become x = s*log2e/8,
    # so exp(s/8) = 2^x for both exp engines.
    qT = np.ascontiguousarray(q.transpose(0, 2, 1) * np.float32(LOG2E / 8))
    qT = qT.astype(np.float16)
    kT = np.ascontiguousarray(k.transpose(0, 2, 1)).astype(np.float16)
    vm1 = np.concatenate([v * mask[:, :, None], mask[:, :, None]], axis=2)
    # second copy pre-scaled by sqrt(1/2): the avg-2 phase-b weight
    # accumulates against it, so the PE sums the two Schraudolph phases.
    vmh = np.concatenate([vm1, vm1 * np.float32(SQRT_HALF)], axis=2)
    vmh = np.ascontiguousarray(vmh).astype(np.float16)

    # Rank-sort batches; slot s takes one batch of rank group [8s, 8s+8)
    # per core, so the baked per-slot tile count wastes little work.
    order = np.argsort(-valid_eff, kind="stable")
    groups = order.reshape(NB, NCORES)[::-1]  # ascending tile counts
    # [S3, S4, S5, S7, S6, S2, S1, S0]: medium slot first (enough exp
    # work to cover the next slot's DMA), the largest batches mid-kernel
    # so the PE gets long continuous stretches (p-state ramp), the two
    # smallest batches last for a short tail.
    perm = [3, 4, 5, NB - 1, NB - 2, 2, 1, 0]
    groups = groups[perm]
    jt_counts = [int(np.ceil(valid_eff[groups[s]].max() / P))
                 for s in range(NB)]
    slots = tuple((jt, int(jt * DVE_FRAC + 0.5))
                  for jt in jt_counts)

    nc = _program_cache.get(slots)
    if nc is None:
        nc = _build_program(slots)
        _program_cache[slots] = nc

    in_maps = []
    for c in range(NCORES):
        bs = groups[:, c]
        in_maps.append({
            "qT": np.ascontiguousarray(qT[bs]),
            "kT": np.ascontiguousarray(kT[bs]),
            "vm": np.ascontiguousarray(vmh[bs]),
        })
    res = bass_utils.run_bass_kernel_spmd(
        nc, in_maps, core_ids=list(range(NCORES)), trace=TRACE,
    )
    LAST_RESULTS = res

    out = np.empty((B, S, D), dtype=np.float32)
    for c in range(NCORES):
        o = res.results[c]["out"]  # [NB, S, W]: numerator + denominator
        on = o[:, :, 0:D] / o[:, :, D:D + 1]
        for s in range(NB):
            out[groups[s, c]] = on[s]
    return out
